# revision 8
# baseline (speedup 1.0000x reference)
"""Per-channel EMA (first-order linear recurrence along time) on 8 TRN2 cores.

  y[b, c, 0] = x[b, c, 0]
  y[b, c, t] = (1 - alpha[c]) * y[b, c, t-1] + alpha[c] * x[b, c, t]

Strategy
  - Data-parallel over batch: B=32 -> 4 batches per core, alpha replicated.
  - Per core: 16 tiles of [128 channels (partitions), 2048 time (free)].
  - The recurrence runs on the DVE via tensor_tensor_scan:
        state = (d * state) + a*x_t,   d = 1 - alpha (per partition)
    with initial = x[:, 0] as a per-partition AP (column 0 needs no special
    case: d*x0 + a*x0 = x0), and d streamed as a stride-0 broadcast AP.
  - The alpha pre-scale (a*x) runs on the Scalar/ACT engine; both compute
    passes hide behind the HBM DMA (memory bound: 32 MiB per core round trip).
  - Queue discipline (Tile emits conservative producer-queue waits, so a
    consumer effectively waits for everything scheduled earlier on the
    producer's queue): the ACT queue carries ONLY prescales so the DVE scan
    chain never waits on a store. Loads and stores share the SP HWDGE queue,
    with loads emitted a few tiles ahead in program order so a store's
    data-ready wait only delays loads that have several tiles of slack.
  - Tile 0 is processed in two chained half-tiles so the scan chain starts
    as soon as the first half-load lands.
"""

import numpy as np

import concourse.bass as bass
import concourse.bacc as bacc
import concourse.mybir as mybir
from concourse.tile import TileContext
from concourse.bass_utils import run_bass_kernel_spmd

B, C, L = 32, 512, 2048
N_CORES = 8
B_SH = B // N_CORES  # 4 batches per core
P = 128              # SBUF partitions
N_CB = C // P        # 4 channel blocks
N_TILES = B_SH * N_CB
LOOKAHEAD = 3        # loads emitted this many tiles ahead of compute+store

_F32 = mybir.dt.float32


def build_nc() -> bass.Bass:
    # Bacc (not raw Bass): its compile() runs generate_event_semaphores,
    # which splits multi-sem waits — TRN2 allows at most one wait command
    # per instruction, and Tile freely emits several.
    nc = bacc.Bacc()
    x = nc.dram_tensor("x", [B_SH, C, L], _F32, kind="ExternalInput")
    alpha = nc.dram_tensor("alpha", [1, C], _F32, kind="ExternalInput")
    y = nc.dram_tensor("y", [B_SH, C, L], _F32, kind="ExternalOutput")

    mult = mybir.AluOpType.mult
    add = mybir.AluOpType.add
    H = L // 2

    with TileContext(nc) as tc:
        with (
            tc.tile_pool(name="xp", bufs=8) as xp,
            tc.tile_pool(name="bp", bufs=6) as bp,
            tc.tile_pool(name="yp", bufs=5) as yp,
            tc.tile_pool(name="cp", bufs=1) as cp,
        ):
            # all 4 channel blocks of alpha in one DMA: [P, N_CB], col j =
            # alpha[j*P + p]
            a4 = cp.tile([P, N_CB], _F32, tag="a4", name="a4")
            nc.sync.dma_start(out=a4, in_=alpha[0].rearrange("(j p) -> p j", j=N_CB))
            d4 = cp.tile([P, N_CB], _F32, tag="d4", name="d4")
            nc.vector.tensor_scalar(
                out=d4, in0=a4, scalar1=-1.0, scalar2=1.0, op0=mult, op1=add
            )

            def tile_cs(n):
                cb, b = divmod(n, B_SH)
                return cb, b, slice(cb * P, (cb + 1) * P)

            x_tiles = {}

            def emit_load(n):
                cb, b, cs = tile_cs(n)
                xt = xp.tile([P, L], _F32, tag="x", name="xt")
                if n == 0:
                    # two half-loads so the first prescale/scan can start on
                    # the first half while the second is still in flight
                    nc.sync.dma_start(out=xt[:, 0:H], in_=x[b, cs, 0:H])
                    nc.sync.dma_start(out=xt[:, H:L], in_=x[b, cs, H:L])
                else:
                    nc.sync.dma_start(out=xt, in_=x[b, cs, :])
                x_tiles[n] = xt

            def emit_compute_store(n):
                cb, b, cs = tile_cs(n)
                xt = x_tiles.pop(n)
                a_ap = a4[:, cb : cb + 1]
                bt = bp.tile([P, L], _F32, tag="b", name="bt")
                yt = yp.tile([P, L], _F32, tag="y", name="yt")
                if n == 0:
                    nc.scalar.mul(bt[:, 0:H], xt[:, 0:H], a_ap)
                    nc.vector.tensor_tensor_scan(
                        out=yt[:, 0:H],
                        data0=d4[:, cb : cb + 1].broadcast_to([P, H]),
                        data1=bt[:, 0:H],
                        initial=xt[:, 0:1],
                        op0=mult,
                        op1=add,
                    )
                    nc.scalar.mul(bt[:, H:L], xt[:, H:L], a_ap)
                    nc.vector.tensor_tensor_scan(
                        out=yt[:, H:L],
                        data0=d4[:, cb : cb + 1].broadcast_to([P, L - H]),
                        data1=bt[:, H:L],
                        initial=yt[:, H - 1 : H],
                        op0=mult,
                        op1=add,
                    )
                else:
                    nc.scalar.mul(bt, xt, a_ap)
                    nc.vector.tensor_tensor_scan(
                        out=yt,
                        data0=d4[:, cb : cb + 1].broadcast_to([P, L]),
                        data1=bt,
                        initial=xt[:, 0:1],
                        op0=mult,
                        op1=add,
                    )
                nc.sync.dma_start(out=y[b, cs, :], in_=yt)

            for n in range(N_TILES + LOOKAHEAD):
                if n < N_TILES:
                    emit_load(n)
                if n >= LOOKAHEAD:
                    emit_compute_store(n - LOOKAHEAD)

    nc.compile()
    return nc


_cached_nc = None


def _get_nc() -> bass.Bass:
    global _cached_nc
    if _cached_nc is None:
        _cached_nc = build_nc()
    return _cached_nc


def kernel(x: np.ndarray, alpha: np.ndarray) -> np.ndarray:
    assert x.shape == (B, C, L) and alpha.shape == (1, C)
    x = np.ascontiguousarray(x, dtype=np.float32)
    alpha = np.ascontiguousarray(alpha, dtype=np.float32)
    nc = _get_nc()
    in_maps = [
        {"x": x[c * B_SH : (c + 1) * B_SH], "alpha": alpha} for c in range(N_CORES)
    ]
    res = run_bass_kernel_spmd(nc, in_maps, list(range(N_CORES)))
    return np.concatenate([r["y"] for r in res.results], axis=0)


# revision 9
# speedup vs baseline: 1.2509x; 1.2509x over previous
"""Per-channel EMA (first-order linear recurrence along time) on 8 TRN2 cores.

  y[b, c, 0] = x[b, c, 0]
  y[b, c, t] = (1 - alpha[c]) * y[b, c, t-1] + alpha[c] * x[b, c, t]

Strategy
  - Data-parallel over batch: B=32 -> 4 batches per core, alpha replicated.
  - Per core: 16 tiles of [128 channels (partitions), 2048 time (free)].
  - The recurrence runs on the DVE via tensor_tensor_scan:
        state = (d * state) + a*x_t,   d = 1 - alpha (per partition)
    with initial = x[:, 0] as a per-partition AP (column 0 needs no special
    case: d*x0 + a*x0 = x0), and d streamed as a stride-0 broadcast AP.
  - The alpha pre-scale (a*x) runs on the Scalar/ACT engine; both compute
    passes hide behind the HBM DMA (memory bound: 32 MiB per core round trip).
  - Queue discipline (Tile emits conservative producer-queue waits, so a
    consumer effectively waits for everything scheduled earlier on the
    producer's queue, and a DMA trigger's wait stalls every trigger behind it
    in the same engine queue):
      * loads alone on the SP HWDGE queue -> they free-run;
      * the ACT queue carries the prescales (always ahead of the scans);
      * stores go through SWDGE on the otherwise-idle Pool engine, except the
        last two which ride the ACT ring - by then the ACT queue is done, and
        HWDGE completion avoids paying the slow SWDGE tail drain for the
        final tile.
  - Tile 0 is processed in two chained half-tiles so the scan chain starts
    as soon as the first half-load lands; a tiny warm-up ACT op pulls the
    activation-table load off the first prescale's critical path.
"""

import numpy as np

import concourse.bass as bass
import concourse.bacc as bacc
import concourse.mybir as mybir
from concourse.tile import TileContext
from concourse.bass_utils import run_bass_kernel_spmd

B, C, L = 32, 512, 2048
N_CORES = 8
B_SH = B // N_CORES  # 4 batches per core
P = 128              # SBUF partitions
N_CB = C // P        # 4 channel blocks
N_TILES = B_SH * N_CB

_F32 = mybir.dt.float32


def build_nc() -> bass.Bass:
    # Bacc (not raw Bass): its compile() runs generate_event_semaphores,
    # which splits multi-sem waits — TRN2 allows at most one wait command
    # per instruction, and Tile freely emits several.
    nc = bacc.Bacc()
    x = nc.dram_tensor("x", [B_SH, C, L], _F32, kind="ExternalInput")
    alpha = nc.dram_tensor("alpha", [1, C], _F32, kind="ExternalInput")
    y = nc.dram_tensor("y", [B_SH, C, L], _F32, kind="ExternalOutput")

    mult = mybir.AluOpType.mult
    add = mybir.AluOpType.add
    H = L // 2

    with TileContext(nc) as tc:
        with (
            tc.tile_pool(name="xp", bufs=6) as xp,
            tc.tile_pool(name="bp", bufs=6) as bp,
            tc.tile_pool(name="yp", bufs=6) as yp,
            tc.tile_pool(name="cp", bufs=1) as cp,
        ):
            # all 4 channel blocks of alpha in one DMA: [P, N_CB], col j =
            # alpha[j*P + p]
            a4 = cp.tile([P, N_CB], _F32, tag="a4", name="a4")
            nc.sync.dma_start(out=a4, in_=alpha[0].rearrange("(j p) -> p j", j=N_CB))
            d4 = cp.tile([P, N_CB], _F32, tag="d4", name="d4")
            nc.vector.tensor_scalar(
                out=d4, in0=a4, scalar1=-1.0, scalar2=1.0, op0=mult, op1=add
            )
            # warm-up ACT op: depends only on the (tiny, early) a4 load, so
            # the framework's ACT_TABLE_LOAD lands before the first real
            # prescale's data arrives
            warm = cp.tile([P, N_CB], _F32, tag="warm", name="warm")
            nc.scalar.mul(warm, a4, 1.0)

            for n in range(N_TILES):
                cb, b = divmod(n, B_SH)
                cs = slice(cb * P, (cb + 1) * P)
                a_ap = a4[:, cb : cb + 1]

                xt = xp.tile([P, L], _F32, tag="x", name="xt")
                bt = bp.tile([P, L], _F32, tag="b", name="bt")
                yt = yp.tile([P, L], _F32, tag="y", name="yt")

                if n == 0:
                    # two chained half-tiles: scan starts on the first half
                    # while the second half is still loading
                    nc.sync.dma_start(out=xt[:, 0:H], in_=x[b, cs, 0:H])
                    nc.sync.dma_start(out=xt[:, H:L], in_=x[b, cs, H:L])
                    nc.scalar.mul(bt[:, 0:H], xt[:, 0:H], a_ap)
                    nc.vector.tensor_tensor_scan(
                        out=yt[:, 0:H],
                        data0=d4[:, cb : cb + 1].broadcast_to([P, H]),
                        data1=bt[:, 0:H],
                        initial=xt[:, 0:1],
                        op0=mult,
                        op1=add,
                    )
                    nc.scalar.mul(bt[:, H:L], xt[:, H:L], a_ap)
                    nc.vector.tensor_tensor_scan(
                        out=yt[:, H:L],
                        data0=d4[:, cb : cb + 1].broadcast_to([P, L - H]),
                        data1=bt[:, H:L],
                        initial=yt[:, H - 1 : H],
                        op0=mult,
                        op1=add,
                    )
                else:
                    nc.sync.dma_start(out=xt, in_=x[b, cs, :])
                    nc.scalar.mul(bt, xt, a_ap)
                    nc.vector.tensor_tensor_scan(
                        out=yt,
                        data0=d4[:, cb : cb + 1].broadcast_to([P, L]),
                        data1=bt,
                        initial=xt[:, 0:1],
                        op0=mult,
                        op1=add,
                    )

                if n >= N_TILES - 2:
                    nc.scalar.dma_start(out=y[b, cs, :], in_=yt)
                else:
                    nc.gpsimd.dma_start(out=y[b, cs, :], in_=yt)

    nc.compile()
    return nc


_cached_nc = None


def _get_nc() -> bass.Bass:
    global _cached_nc
    if _cached_nc is None:
        _cached_nc = build_nc()
    return _cached_nc


def kernel(x: np.ndarray, alpha: np.ndarray) -> np.ndarray:
    assert x.shape == (B, C, L) and alpha.shape == (1, C)
    x = np.ascontiguousarray(x, dtype=np.float32)
    alpha = np.ascontiguousarray(alpha, dtype=np.float32)
    nc = _get_nc()
    in_maps = [
        {"x": x[c * B_SH : (c + 1) * B_SH], "alpha": alpha} for c in range(N_CORES)
    ]
    res = run_bass_kernel_spmd(nc, in_maps, list(range(N_CORES)))
    return np.concatenate([r["y"] for r in res.results], axis=0)
